# revision 3
# baseline (speedup 1.0000x reference)
"""Causal self-attention on 8 TRN2 NeuronCores.

Sharding: core c -> (batch b = c//2, head-group g = c%2). Each core computes
attention for 8 of the 16 heads of one batch element plus its half of the
output projection; the host sums the two head-group partials per batch.

Device layout notes:
  - All matmuls run in float32r (full PE rate at N>=512, ~1.5e-4 rel err).
  - Scores are computed transposed (S^T[j, i], keys on partitions) so softmax
    exp output feeds the AV matmul with no transposes anywhere.
  - Softmax skips max-subtraction (scores are O(1) for this problem) and the
    denominator is produced by a ones-column appended to v in the AV matmul.
"""
import numpy as np

B, T, D = 4, 2048, 1024
NH_LOCAL = 8          # heads per core
HD = 64               # head dim
CL = 512              # local channels = NH_LOCAL * HD
P = 128
CC = D // P           # 8 contraction chunks
TC = T // P           # 16 t-chunks (key blocks)
TT = T // 512         # 4 t-tiles
NPAIR = 4             # head pairs per core

_CACHE = {}


def _build():
    import concourse.bacc as bacc
    import concourse.mybir as mybir
    import concourse.tile as tile
    from contextlib import ExitStack

    f32 = mybir.dt.float32
    f32r = mybir.dt.float32r
    EXP = mybir.ActivationFunctionType.Exp
    MULT = mybir.AluOpType.mult

    nc = bacc.Bacc("TRN2", target_bir_lowering=False, debug=False)

    xT = nc.dram_tensor("xT", (D, T), f32r, kind="ExternalInput")
    wqT = nc.dram_tensor("wqT", (D, CL), f32r, kind="ExternalInput")
    wkT = nc.dram_tensor("wkT", (D, CL), f32r, kind="ExternalInput")
    wvT = nc.dram_tensor("wvT", (D, CL), f32r, kind="ExternalInput")
    woT = nc.dram_tensor("woT", (CL, D), f32r, kind="ExternalInput")
    tri = nc.dram_tensor("tri", (P, P), f32, kind="ExternalInput")
    yT = nc.dram_tensor("yT", (D, T), f32, kind="ExternalOutput")

    xT_r = xT.ap().rearrange("(o p) t -> p o t", p=P)       # [128, 8, 2048]
    wqT_r = wqT.ap().rearrange("(o p) f -> p o f", p=P)     # [128, 8, 512]
    wkT_r = wkT.ap().rearrange("(o p) f -> p o f", p=P)
    wvT_r = wvT.ap().rearrange("(o p) f -> p o f", p=P)
    woT_r = woT.ap().rearrange("(o p) f -> p o f", p=P)     # [128, 4, 1024]
    yT_r = yT.ap().rearrange("(o p) t -> p o t", p=P)       # [128, 8, 2048]

    with tile.TileContext(nc) as tc, ExitStack() as outer:
        # ---- resident tiles (whole kernel) ----
        persist = outer.enter_context(tc.tile_pool(name="persist", bufs=1))
        qT_sb = persist.tile([P, NPAIR, T], f32r, tag="qT")
        kT_sb = persist.tile([P, NPAIR, T], f32r, tag="kT")
        v_sb = persist.tile([P, TC, NH_LOCAL, HD + 1], f32r, tag="v")
        tri_sb = persist.tile([P, P], f32, tag="tri")
        nc.sync.dma_start(tri_sb[:], tri.ap())
        # ones column for the softmax-denominator trick
        nc.vector.memset(v_sb[:, :, :, HD:HD + 1].bitcast(f32), 1.0)

        # ================= Phase 1: QKV projections =================
        with tc.tile_pool(name="p1x", bufs=1) as p1x, \
             tc.tile_pool(name="p1ps", bufs=2, space="PSUM") as p1ps:
            xT_sb = p1x.tile([P, CC, T], f32r, tag="xT")
            nc.sync.dma_start(xT_sb[:], xT_r)

            # v = x @ wv^T in [t, c_local] layout, all heads at once (N=512)
            with tc.tile_pool(name="p1wv", bufs=1) as p1wv:
                wv_sb = p1wv.tile([P, CC, CL], f32r, tag="wv")
                nc.sync.dma_start(wv_sb[:], wvT_r)
                for t_c in range(TC):
                    pv = p1ps.tile([P, CL], f32, tag="pv")
                    for cc in range(CC):
                        nc.tensor.matmul(
                            pv[:],
                            xT_sb[:, cc, t_c * P:(t_c + 1) * P],
                            wv_sb[:, cc, :],
                            start=(cc == 0), stop=(cc == CC - 1))
                    nc.vector.tensor_copy(
                        v_sb[:, t_c, :, 0:HD],
                        pv[:].rearrange("p (h d) -> p h d", h=NH_LOCAL))

            # qT / kT in [c_local, t] layout, per head pair (f slice of 128)
            with tc.tile_pool(name="p1w", bufs=2) as p1w:
                for p_i in range(NPAIR):
                    for w_r, dst, wtag in ((wqT_r, qT_sb, "wq"),
                                           (wkT_r, kT_sb, "wk")):
                        w_sl = p1w.tile([P, CC, P], f32r, tag=wtag)
                        nc.sync.dma_start(
                            w_sl[:], w_r[:, :, p_i * P:(p_i + 1) * P])
                        for half in range(2):
                            pq = p1ps.tile([P, 1024], f32, tag="pq")
                            for s5 in range(2):
                                for cc in range(CC):
                                    nc.tensor.matmul(
                                        pq[:, s5 * 512:(s5 + 1) * 512],
                                        w_sl[:, cc, :],
                                        xT_sb[:, cc,
                                              half * 1024 + s5 * 512:
                                              half * 1024 + (s5 + 1) * 512],
                                        start=(cc == 0), stop=(cc == CC - 1))
                            nc.vector.tensor_copy(
                                dst[:, p_i, half * 1024:(half + 1) * 1024],
                                pq[:])

        # ================= Phases 2+3 share the aT pool =================
        with tc.tile_pool(name="aT", bufs=1) as aTp:
            aT_sb = aTp.tile([P, NPAIR, T], f32r, tag="aT")
            self_attention(nc, tc, tile, mybir, qT_sb, kT_sb, v_sb, tri_sb,
                           aT_sb)
            out_proj(nc, tc, tile, mybir, woT_r, yT_r, aT_sb)

    nc.compile()
    return nc


def self_attention(nc, tc, tile, mybir, qT_sb, kT_sb, v_sb, tri_sb, aT_sb):
    f32 = mybir.dt.float32
    f32r = mybir.dt.float32r
    EXP = mybir.ActivationFunctionType.Exp
    MULT = mybir.AluOpType.mult
    if True:
        with tc.tile_pool(name="p2", bufs=2) as p2, \
             tc.tile_pool(name="p2pt", bufs=2) as p2pt, \
             tc.tile_pool(name="p2d", bufs=4, space="DRAM") as p2d, \
             tc.tile_pool(name="p2aps", bufs=1, space="PSUM") as p2aps, \
             tc.tile_pool(name="p2sps", bufs=2, space="PSUM") as p2sps:
            for h in range(NH_LOCAL):
                p_i, par = h // 2, h % 2
                prow = 64 * par
                aT_ps = p2aps.tile([HD + 1, T], f32, tag="aT_ps")
                for jc in range(TC):
                    win0 = 512 * (jc // 4)
                    nwin = T - win0
                    off = P * (jc % 4)
                    pt = p2pt.tile([P, T], f32r, tag="pt")
                    if off:
                        nc.vector.memset(pt[:, :off].bitcast(f32), 0.0)
                    for chs, chunk_start in enumerate(range(win0, T, 1024)):
                        clen = min(1024, T - chunk_start)
                        st = p2sps.tile([P, 1024], f32, tag="st")
                        for s5 in range(0, clen, 512):
                            nc.tensor.matmul(
                                st[:, s5:s5 + 512],
                                kT_sb[prow:prow + HD, p_i,
                                      jc * P:(jc + 1) * P],
                                qT_sb[prow:prow + HD, p_i,
                                      chunk_start + s5:chunk_start + s5 + 512],
                                start=True, stop=True)
                        eoff = off if chs == 0 else 0
                        nc.scalar.activation(
                            pt[:, chunk_start - win0 + eoff:
                               chunk_start - win0 + clen],
                            st[:, eoff:clen], EXP, scale=0.125)
                        if chs == 0:
                            nc.vector.tensor_tensor(
                                pt[:, off:off + P], pt[:, off:off + P],
                                tri_sb[:], MULT)
                        for s5 in range(0, clen, 512):
                            i0 = chunk_start + s5
                            it = i0 // 512
                            nc.tensor.matmul(
                                aT_ps[:, i0:i0 + 512],
                                v_sb[:, jc, h, :],
                                pt[:, i0 - win0:i0 - win0 + 512],
                                start=(jc == 0), stop=(jc == 4 * it + 3))
                    # normalize i-tiles that just completed (it == jc - 3 .. )
                    if jc % 4 == 3:
                        it = jc // 4
                        i0 = it * 512
                        rr = p2.tile([P, 512], f32, tag="rr")
                        nc.vector.reciprocal(
                            rr[64:65, :], aT_ps[64:65, i0:i0 + 512])
                        dtmp = p2d.tile([512], f32, tag="dtmp")
                        nc.sync.dma_start(dtmp[:], rr[64:65, :])
                        rb = p2.tile([HD, 512], f32, tag="rb")
                        nc.sync.dma_start(
                            rb[:], dtmp[None, :].to_broadcast((HD, 512)))
                        if par == 0:
                            nc.vector.tensor_tensor(
                                aT_sb[0:HD, p_i, i0:i0 + 512],
                                aT_ps[0:HD, i0:i0 + 512], rb[:], MULT)
                        else:
                            t64 = p2.tile([HD, 512], f32r, tag="t64")
                            nc.vector.tensor_tensor(
                                t64[:], aT_ps[0:HD, i0:i0 + 512], rb[:], MULT)
                            nc.sync.dma_start(
                                aT_sb[HD:P, p_i, i0:i0 + 512], t64[:])



def out_proj(nc, tc, tile, mybir, woT_r, yT_r, aT_sb):
    f32 = mybir.dt.float32
    f32r = mybir.dt.float32r
    with tc.tile_pool(name="p3", bufs=4) as p3, \
         tc.tile_pool(name="p3w", bufs=1) as p3w, \
         tc.tile_pool(name="p3ps", bufs=4, space="PSUM") as p3ps:
        wo_sb = p3w.tile([P, NPAIR, D], f32r, tag="wo")
        nc.sync.dma_start(wo_sb[:], woT_r)
        for fc in range(CC):
            for tt in range(TT):
                py = p3ps.tile([P, 512], f32, tag="py")
                for cc in range(NPAIR):
                    nc.tensor.matmul(
                        py[:],
                        wo_sb[:, cc, fc * P:(fc + 1) * P],
                        aT_sb[:, cc, tt * 512:(tt + 1) * 512],
                        start=(cc == 0), stop=(cc == NPAIR - 1))
                yst = p3.tile([P, 512], f32, tag="yst")
                nc.vector.tensor_copy(yst[:], py[:])
                nc.sync.dma_start(
                    yT_r[:, fc, tt * 512:(tt + 1) * 512], yst[:])


def kernel(x, w_qkv, w_out):
    from concourse import bass_utils

    if "nc" not in _CACHE:
        _CACHE["nc"] = _build()
    nc = _CACHE["nc"]

    x = np.asarray(x, dtype=np.float32)
    w_qkv = np.asarray(w_qkv, dtype=np.float32)
    w_out = np.asarray(w_out, dtype=np.float32)
    tri = np.triu(np.ones((P, P), dtype=np.float32))

    in_maps = []
    for c in range(8):
        b, g = c // 2, c % 2
        sl = slice(CL * g, CL * g + CL)
        in_maps.append({
            "xT": np.ascontiguousarray(x[b].T),
            "wqT": np.ascontiguousarray(w_qkv[0 * D:1 * D][sl].T),
            "wkT": np.ascontiguousarray(w_qkv[1 * D:2 * D][sl].T),
            "wvT": np.ascontiguousarray(w_qkv[2 * D:3 * D][sl].T),
            "woT": np.ascontiguousarray(w_out[:, sl].T),
            "tri": tri,
        })

    res = bass_utils.run_bass_kernel_spmd(nc, in_maps, core_ids=list(range(8)))
    outs = res.results

    y = np.empty((B, T, D), dtype=np.float32)
    for b in range(B):
        y[b] = (outs[2 * b]["yT"] + outs[2 * b + 1]["yT"]).T
    return y


# revision 4
# speedup vs baseline: 166.2715x; 166.2715x over previous
"""Causal self-attention on 8 TRN2 NeuronCores.

Sharding: core c -> (batch b = c//2, head-group g = c%2). Each core computes
attention for 8 of the 16 heads of one batch element plus its half of the
output projection; the host sums the two head-group partials per batch.

Device layout notes:
  - All matmuls run in float32r (full PE rate at N>=512, ~1.5e-4 rel err).
  - Scores are computed transposed (S^T[j, i], keys on partitions) so softmax
    exp output feeds the AV matmul with no transposes anywhere.
  - Softmax skips max-subtraction (scores are O(1) for this problem) and the
    denominator is produced by a ones-column appended to v in the AV matmul.
"""
import numpy as np

B, T, D = 4, 2048, 1024
NH_LOCAL = 8          # heads per core
HD = 64               # head dim
CL = 512              # local channels = NH_LOCAL * HD
P = 128
CC = D // P           # 8 contraction chunks
TC = T // P           # 16 t-chunks (key blocks)
TT = T // 512         # 4 t-tiles
NPAIR = 4             # head pairs per core

_CACHE = {}


def _emit_qkv(nc, tc, mybir, r, aps, qT_sb, kT_sb, v_sb):
    f32 = mybir.dt.float32
    f32r = mybir.dt.float32r
    xT_r, wqT_r, wkT_r, wvT_r = aps
    with tc.tile_pool(name=f"p1x{r}", bufs=1) as p1x, \
         tc.tile_pool(name=f"p1ps{r}", bufs=2, space="PSUM") as p1ps:
        xT_sb = p1x.tile([P, CC, T], f32r, tag="xT")
        for cc in range(CC):
            nc.sync.dma_start(xT_sb[:, cc, :], xT_r[:, cc, :])

        # v = x @ wv^T in [t, c_local] layout, all heads at once (N=512)
        with tc.tile_pool(name=f"p1wv{r}", bufs=1) as p1wv:
            wv_sb = p1wv.tile([P, CC, CL], f32r, tag="wv")
            for cc in range(CC):
                nc.sync.dma_start(wv_sb[:, cc, :], wvT_r[:, cc, :])
            for t_c in range(TC):
                pv = p1ps.tile([P, CL], f32, tag="pv")
                for cc in range(CC):
                    nc.tensor.matmul(
                        pv[:],
                        xT_sb[:, cc, t_c * P:(t_c + 1) * P],
                        wv_sb[:, cc, :],
                        start=(cc == 0), stop=(cc == CC - 1))
                nc.vector.tensor_copy(
                    v_sb[:, t_c, :, 0:HD],
                    pv[:].rearrange("p (h d) -> p h d", h=NH_LOCAL))

        # qT / kT in [c_local, t] layout, per head pair (f slice of 128)
        with tc.tile_pool(name=f"p1w{r}", bufs=2) as p1w:
            for p_i in range(NPAIR):
                for w_r, dst, wtag in ((wqT_r, qT_sb, "wq"),
                                       (wkT_r, kT_sb, "wk")):
                    w_sl = p1w.tile([P, CC, P], f32r, tag=wtag)
                    nc.sync.dma_start(
                        w_sl[:], w_r[:, :, p_i * P:(p_i + 1) * P])
                    for half in range(2):
                        pq = p1ps.tile([P, 1024], f32, tag="pq")
                        for s5 in range(2):
                            for cc in range(CC):
                                nc.tensor.matmul(
                                    pq[:, s5 * 512:(s5 + 1) * 512],
                                    w_sl[:, cc, :],
                                    xT_sb[:, cc,
                                          half * 1024 + s5 * 512:
                                          half * 1024 + (s5 + 1) * 512],
                                    start=(cc == 0), stop=(cc == CC - 1))
                        nc.vector.tensor_copy(
                            dst[:, p_i, half * 1024:(half + 1) * 1024],
                            pq[:])


def _emit_attention(nc, tc, mybir, r, qT_sb, kT_sb, v_sb, tri_sb, aT_sb):
    f32 = mybir.dt.float32
    f32r = mybir.dt.float32r
    EXP = mybir.ActivationFunctionType.Exp
    MULT = mybir.AluOpType.mult
    with tc.tile_pool(name=f"p2{r}", bufs=2) as p2, \
         tc.tile_pool(name=f"p2pt{r}", bufs=2) as p2pt, \
         tc.tile_pool(name=f"p2d{r}", bufs=4, space="DRAM") as p2d, \
         tc.tile_pool(name=f"p2aps{r}", bufs=1, space="PSUM") as p2aps, \
         tc.tile_pool(name=f"p2sps{r}", bufs=2, space="PSUM") as p2sps:
        for h in range(NH_LOCAL):
            p_i, par = h // 2, h % 2
            prow = 64 * par
            aT_ps = p2aps.tile([HD + 1, T], f32, tag="aT_ps")
            for jc in range(TC):
                win0 = 512 * (jc // 4)
                off = P * (jc % 4)
                pt = p2pt.tile([P, T], f32r, tag="pt")
                if off:
                    nc.vector.memset(pt[:, :off].bitcast(f32), 0.0)
                for chs, chunk_start in enumerate(range(win0, T, 1024)):
                    clen = min(1024, T - chunk_start)
                    st = p2sps.tile([P, 1024], f32, tag="st")
                    for s5 in range(0, clen, 512):
                        nc.tensor.matmul(
                            st[:, s5:s5 + 512],
                            kT_sb[prow:prow + HD, p_i,
                                  jc * P:(jc + 1) * P],
                            qT_sb[prow:prow + HD, p_i,
                                  chunk_start + s5:chunk_start + s5 + 512],
                            start=True, stop=True)
                    eoff = off if chs == 0 else 0
                    nc.scalar.activation(
                        pt[:, chunk_start - win0 + eoff:
                           chunk_start - win0 + clen],
                        st[:, eoff:clen], EXP, scale=0.125)
                    if chs == 0:
                        nc.vector.tensor_tensor(
                            pt[:, off:off + P], pt[:, off:off + P],
                            tri_sb[:], MULT)
                    for s5 in range(0, clen, 512):
                        i0 = chunk_start + s5
                        it = i0 // 512
                        nc.tensor.matmul(
                            aT_ps[:, i0:i0 + 512],
                            v_sb[:, jc, h, :],
                            pt[:, i0 - win0:i0 - win0 + 512],
                            start=(jc == 0), stop=(jc == 4 * it + 3))
                # normalize the i-tile whose accumulation just finished
                if jc % 4 == 3:
                    i0 = (jc // 4) * 512
                    rr = p2.tile([P, 512], f32, tag="rr")
                    nc.vector.reciprocal(
                        rr[64:65, :], aT_ps[64:65, i0:i0 + 512])
                    dtmp = p2d.tile([512], f32, tag="dtmp")
                    nc.sync.dma_start(dtmp[:], rr[64:65, :])
                    rb = p2.tile([HD, 512], f32, tag="rb")
                    nc.sync.dma_start(
                        rb[:], dtmp[None, :].to_broadcast((HD, 512)))
                    if par == 0:
                        nc.vector.tensor_tensor(
                            aT_sb[0:HD, p_i, i0:i0 + 512],
                            aT_ps[0:HD, i0:i0 + 512], rb[:], MULT)
                    else:
                        t64 = p2.tile([HD, 512], f32r, tag="t64")
                        nc.vector.tensor_tensor(
                            t64[:], aT_ps[0:HD, i0:i0 + 512], rb[:], MULT)
                        nc.sync.dma_start(
                            aT_sb[HD:P, p_i, i0:i0 + 512], t64[:])


def _emit_out_proj(nc, tc, mybir, r, woT_r, yT_r, aT_sb):
    f32 = mybir.dt.float32
    f32r = mybir.dt.float32r
    with tc.tile_pool(name=f"p3{r}", bufs=4) as p3, \
         tc.tile_pool(name=f"p3w{r}", bufs=1) as p3w, \
         tc.tile_pool(name=f"p3ps{r}", bufs=4, space="PSUM") as p3ps:
        wo_sb = p3w.tile([P, NPAIR, D], f32r, tag="wo")
        nc.sync.dma_start(wo_sb[:], woT_r)
        for fc in range(CC):
            for tt in range(TT):
                py = p3ps.tile([P, 512], f32, tag="py")
                for cc in range(NPAIR):
                    nc.tensor.matmul(
                        py[:],
                        wo_sb[:, cc, fc * P:(fc + 1) * P],
                        aT_sb[:, cc, tt * 512:(tt + 1) * 512],
                        start=(cc == 0), stop=(cc == NPAIR - 1))
                yst = p3.tile([P, 512], f32, tag="yst")
                nc.vector.tensor_copy(yst[:], py[:])
                nc.sync.dma_start(
                    yT_r[:, fc, tt * 512:(tt + 1) * 512], yst[:])


def _build(repeats=1):
    import concourse.bacc as bacc
    import concourse.mybir as mybir
    import concourse.tile as tile
    from contextlib import ExitStack

    f32 = mybir.dt.float32
    f32r = mybir.dt.float32r

    nc = bacc.Bacc("TRN2", target_bir_lowering=False, debug=False)

    xT = nc.dram_tensor("xT", (D, T), f32r, kind="ExternalInput")
    wqT = nc.dram_tensor("wqT", (D, CL), f32r, kind="ExternalInput")
    wkT = nc.dram_tensor("wkT", (D, CL), f32r, kind="ExternalInput")
    wvT = nc.dram_tensor("wvT", (D, CL), f32r, kind="ExternalInput")
    woT = nc.dram_tensor("woT", (CL, D), f32r, kind="ExternalInput")
    tri = nc.dram_tensor("tri", (P, P), f32, kind="ExternalInput")
    yT = nc.dram_tensor("yT", (D, T), f32, kind="ExternalOutput")

    xT_r = xT.ap().rearrange("(o p) t -> p o t", p=P)       # [128, 8, 2048]
    wqT_r = wqT.ap().rearrange("(o p) f -> p o f", p=P)     # [128, 8, 512]
    wkT_r = wkT.ap().rearrange("(o p) f -> p o f", p=P)
    wvT_r = wvT.ap().rearrange("(o p) f -> p o f", p=P)
    woT_r = woT.ap().rearrange("(o p) f -> p o f", p=P)     # [128, 4, 1024]
    yT_r = yT.ap().rearrange("(o p) t -> p o t", p=P)       # [128, 8, 2048]

    with tile.TileContext(nc) as tc, ExitStack() as outer:
        persist = outer.enter_context(tc.tile_pool(name="persist", bufs=1))
        qT_sb = persist.tile([P, NPAIR, T], f32r, tag="qT")
        kT_sb = persist.tile([P, NPAIR, T], f32r, tag="kT")
        v_sb = persist.tile([P, TC, NH_LOCAL, HD + 1], f32r, tag="v")
        tri_sb = persist.tile([P, P], f32, tag="tri")
        nc.sync.dma_start(tri_sb[:], tri.ap())

        for r in range(repeats):
            # ones column for the softmax-denominator trick
            nc.vector.memset(v_sb[:, :, :, HD:HD + 1].bitcast(f32), 1.0)
            _emit_qkv(nc, tc, mybir, r, (xT_r, wqT_r, wkT_r, wvT_r),
                      qT_sb, kT_sb, v_sb)
            with tc.tile_pool(name=f"aT{r}", bufs=1) as aTp:
                aT_sb = aTp.tile([P, NPAIR, T], f32r, tag="aT")
                _emit_attention(nc, tc, mybir, r, qT_sb, kT_sb, v_sb,
                                tri_sb, aT_sb)
                _emit_out_proj(nc, tc, mybir, r, woT_r, yT_r, aT_sb)

    nc.compile()
    return nc


def kernel(x, w_qkv, w_out):
    from concourse import bass_utils

    if "nc" not in _CACHE:
        _CACHE["nc"] = _build()
    nc = _CACHE["nc"]

    x = np.asarray(x, dtype=np.float32)
    w_qkv = np.asarray(w_qkv, dtype=np.float32)
    w_out = np.asarray(w_out, dtype=np.float32)
    tri = np.triu(np.ones((P, P), dtype=np.float32))

    in_maps = []
    for c in range(8):
        b, g = c // 2, c % 2
        sl = slice(CL * g, CL * g + CL)
        in_maps.append({
            "xT": np.ascontiguousarray(x[b].T),
            "wqT": np.ascontiguousarray(w_qkv[0 * D:1 * D][sl].T),
            "wkT": np.ascontiguousarray(w_qkv[1 * D:2 * D][sl].T),
            "wvT": np.ascontiguousarray(w_qkv[2 * D:3 * D][sl].T),
            "woT": np.ascontiguousarray(w_out[:, sl].T),
            "tri": tri,
        })

    res = bass_utils.run_bass_kernel_spmd(nc, in_maps, core_ids=list(range(8)))
    outs = res.results

    y = np.empty((B, T, D), dtype=np.float32)
    for b in range(B):
        y[b] = (outs[2 * b]["yT"] + outs[2 * b + 1]["yT"]).T
    return y
